# revision 8
# baseline (speedup 1.0000x reference)
"""Causal multi-head attention block (qkv proj -> causal softmax attention ->
out proj) for Trainium2, distributed over 8 NeuronCores.

Sharding: 4-way data parallel over batch x 2-way tensor parallel over heads
(8 heads per core). Each core computes, for its (batch, head-group):
  - QKV projection directly in transposed-per-head layout (Q^T/K^T [hd, T])
  - causal softmax attention entirely in the "scores transposed" [k, q]
    orientation (row-sums via an appended ones-column in the PV matmul)
  - its partial output projection (row-parallel W_proj split)
Host side packs/shards inputs, and sums the two head-group partials per batch.

All matmuls run as float32r (TF32-like, 1 cycle/row at free-dim >= 256).
"""
import numpy as np
from contextlib import ExitStack

import concourse.bacc as bacc
import concourse.tile as tile
from concourse import mybir
from concourse.alu_op_type import AluOpType
from concourse.bass_utils import run_bass_kernel_spmd

F32 = mybir.dt.float32
F32R = mybir.dt.float32r
EXP = mybir.ActivationFunctionType.Exp

T, C, HD, HL = 2048, 1024, 64, 8   # seq, d_model, head_dim, heads-per-core
N_CORES = 8

_CACHE = {}
last_results = None   # BassKernelResults of the most recent run (for test.py)


def _build_nc():
    nc = bacc.Bacc("TRN2", target_bir_lowering=False, debug=False,
                   num_devices=N_CORES)
    xT = nc.dram_tensor("xT", [C, T], F32R, kind="ExternalInput").ap()
    wqk = nc.dram_tensor("wqk", [C, 1024], F32R, kind="ExternalInput").ap()
    wv = nc.dram_tensor("wv", [C, 512], F32R, kind="ExternalInput").ap()
    wp = nc.dram_tensor("wp", [512, C], F32R, kind="ExternalInput").ap()
    bqk = nc.dram_tensor("bqk", [128, 8], F32, kind="ExternalInput").ap()
    bo = nc.dram_tensor("bo", [128, 8], F32, kind="ExternalInput").ap()
    tri = nc.dram_tensor("trimask", [128, 128], F32R, kind="ExternalInput").ap()
    vones = nc.dram_tensor("vones", [128, 32], F32R, kind="ExternalInput").ap()
    outT = nc.dram_tensor("outT", [C, T], F32, kind="ExternalOutput").ap()

    with tile.TileContext(nc) as tc, ExitStack() as ctx:
        def pool(name, bufs, space="SBUF"):
            return ctx.enter_context(tc.tile_pool(name=name, bufs=bufs, space=space))

        constp = pool("const", 1)
        wqkp = pool("wqkp", 1)
        wvp = pool("wvp", 1)
        ytp = pool("ytp", 1)
        vp = pool("vp", 3)
        qkp = pool("qkp", 2)
        bcp = pool("bcp", 1)
        ptp = pool("ptp", 2)
        osbp = pool("osbp", 2)
        rpp = pool("rpp", 2)
        pp = pool("pp", 2, "PSUM")
        stp = pool("stp", 2, "PSUM")
        yp = pool("yp", 2, "PSUM")
        # opened last so it can be released (LIFO) right after the final
        # projection, freeing its 64KB/partition for the wp tile
        xtp_cm = tc.tile_pool(name="xtp", bufs=1)
        xtp = xtp_cm.__enter__()

        bqk_t = constp.tile([128, 8], F32)
        nc.sync.dma_start(out=bqk_t, in_=bqk)
        # tiny warmup exp so the ACT table-set DMA happens during the input
        # loads instead of stalling the first real softmax tile
        warm = constp.tile([1, 8], F32)
        nc.scalar.activation(warm[0:1, :], bqk_t[0:1, :], EXP)
        bo_t = constp.tile([128, 8], F32)
        nc.sync.dma_start(out=bo_t, in_=bo)
        tri_t = constp.tile([128, 128], F32R)
        nc.sync.dma_start(out=tri_t, in_=tri)

        wqkr = wqk.rearrange("(c p) m -> p c m", p=128)   # [128, 8, 1024]
        wvr = wv.rearrange("(c p) m -> p c m", p=128)     # [128, 8, 512]
        wpr = wp.rearrange("(c p) m -> p c m", p=128)     # [128, 4, 1024]
        # first pair's weights land before the bulk x load so the PE can
        # start projecting as soon as x's first t-block arrives
        wqk_pr0 = wqkp.tile([128, 8, 256], F32R, name="wqkpr")
        nc.sync.dma_start(out=wqk_pr0, in_=wqkr[:, :, 0:256])
        wv_g0 = wvp.tile([128, 8, 256], F32R, name="wvg")
        nc.sync.dma_start(out=wv_g0, in_=wvr[:, :, 0:256])
        # x^T resident: [128, c-chunk, t-block, 512]
        xt = xtp.tile([128, 8, 4, 512], F32R)
        xTr = xT.rearrange("(c p) (tb n) -> p c tb n", p=128, n=512)
        for tb in range(4):
            for c in range(8):
                nc.sync.dma_start(out=xt[:, c, tb], in_=xTr[:, c, tb])

        # y^T staging, one tile per t-block so the out-projection's reads
        # only depend on that t-block's writers: each [128(pair rows), pair, 512]
        yts = [ytp.tile([128, 4, 512], F32R, name=f"yt{tb}") for tb in range(4)]

        v_tiles = {}
        qk_tiles = {}

        def v_group_proj(g):
            """Project V for head pairs 2g, 2g+1 (4 heads, N=256 per matmul)."""
            if g == 0:
                wv_g = wv_g0
            else:
                wv_g = wvp.tile([128, 8, 256], F32R, name="wvg")
                nc.sync.dma_start(out=wv_g, in_=wvr[:, :, g * 256:(g + 1) * 256])
            for prl in (2 * g, 2 * g + 1):
                vt = vp.tile([128, 2, 16, 65], F32R, name="v")
                v_tiles[prl] = vt
                # rowsum ones-column (col 64 of every k-tile)
                nc.sync.dma_start(
                    out=vt[:, 0:2, :, 64:65],
                    in_=vones.rearrange("p (a kt one) -> p a kt one", a=2, one=1))
            for tb in range(4):
                for tt in range(4):
                    p = pp.tile([128, 256], F32, name="pp")
                    for c in range(8):
                        nc.tensor.matmul(p, lhsT=xt[:, c, tb, tt * 128:(tt + 1) * 128],
                                         rhs=wv_g[:, c, :],
                                         start=(c == 0), stop=(c == 7))
                    kt = tb * 4 + tt
                    for j, prl in enumerate((2 * g, 2 * g + 1)):
                        src = p[:, j * 128:(j + 1) * 128].rearrange(
                            "p (a d) -> p a d", a=2)
                        nc.vector.tensor_copy(
                            out=v_tiles[prl][:, 0:2, kt, 0:64], in_=src)

        def qk_proj(pr):
            """Q^T/K^T for pair pr: psum rows = head A (0:64) | head B (64:128)."""
            if pr == 0:
                wqk_pr = wqk_pr0
            else:
                wqk_pr = wqkp.tile([128, 8, 256], F32R, name="wqkpr")
                nc.sync.dma_start(out=wqk_pr,
                                  in_=wqkr[:, :, 2 * pr * 128:(2 * pr + 2) * 128])
            qk = qkp.tile([128, 2, T], F32R, name="qk")
            qk_tiles[pr] = qk
            for tb in range(4):
                for mi in range(2):       # 0 = QQ, 1 = KK
                    p = pp.tile([128, 512], F32, name="pp")
                    for c in range(8):
                        nc.tensor.matmul(p,
                                         lhsT=wqk_pr[:, c, mi * 128:(mi + 1) * 128],
                                         rhs=xt[:, c, tb, :],
                                         start=(c == 0), stop=(c == 7))
                    nc.vector.tensor_scalar_add(
                        out=qk[:, mi, tb * 512:(tb + 1) * 512], in0=p,
                        scalar1=bqk_t[:, 2 * pr + mi: 2 * pr + mi + 1])

        def attention(pr):
            qk = qk_tiles[pr]
            vt = v_tiles[pr]
            for qb in range(4):
                K = 4 * qb + 4
                ys = [yp.tile([65, 512], F32, name="y") for _ in range(2)]
                for kt in range(K):
                    d = kt - 4 * qb
                    lo_mm = 0 if d < 0 else min(128 * d, 256)
                    lo_pv = 0 if d < 0 else 128 * d
                    st = stp.tile([128, 1024], F32, name="st")
                    for a in range(2):
                        po = a * 64
                        # S^T[k, q] = K^T_chunk.T @ Q^T ; heads A/B row-packed
                        nc.tensor.matmul(
                            st[:, a * 512 + lo_mm:(a + 1) * 512],
                            lhsT=qk[po:po + 64, 1, kt * 128:(kt + 1) * 128],
                            rhs=qk[po:po + 64, 0,
                                   qb * 512 + lo_mm:(qb + 1) * 512],
                            start=True, stop=True,
                            tile_position=(po, 0))
                    pt = ptp.tile([128, 1024], F32R, name="pt")
                    nc.scalar.activation(pt[:, lo_mm:1024], st[:, lo_mm:1024], EXP)
                    if d >= 0:
                        for a in range(2):
                            blk = slice(a * 512 + 128 * d, a * 512 + 128 * d + 128)
                            nc.vector.tensor_tensor(out=pt[:, blk], in0=pt[:, blk],
                                                    in1=tri_t, op=AluOpType.mult)
                    for a in range(2):
                        nc.tensor.matmul(
                            ys[a][:, lo_pv:512],
                            lhsT=vt[:, a, kt, :],
                            rhs=pt[:, a * 512 + lo_pv:(a + 1) * 512],
                            start=(kt == 0), stop=(kt == K - 1))
                for a in range(2):
                    rs = rpp.tile([1, 512], F32, name="rs")
                    nc.vector.tensor_copy(out=rs[0:1, :], in_=ys[a][64:65, :])
                    rc = rpp.tile([1, 512], F32, name="rc")
                    nc.vector.reciprocal_approx_fast(out=rc[0:1, :],
                                                     in_=rs[0:1, :])
                    bct = bcp.tile([64, 512], F32, name="bc")
                    nc.gpsimd.partition_broadcast(bct, rc[0:1, :], channels=64)
                    nc.vector.tensor_tensor(
                        out=yts[qb][a * 64:(a + 1) * 64, pr, :],
                        in0=ys[a][0:64, :], in1=bct, op=AluOpType.mult)

        qk_proj(0)
        v_group_proj(0)
        attention(0)
        qk_proj(1)
        v_group_proj(1)
        attention(1)
        qk_proj(2)
        attention(2)
        qk_proj(3)
        xtp_cm.__exit__(None, None, None)   # frees x for the wp tile below
        attention(3)

        wpfp = ctx.enter_context(tc.tile_pool(name="wpfp", bufs=1))
        wp_t = wpfp.tile([128, 4, 1024], F32R, name="wpfull")
        nc.sync.dma_start(out=wp_t, in_=wpr)
        for tb in range(4):
            for oc in range(8):
                p = pp.tile([128, 512], F32, name="pp")
                for yc in range(4):
                    nc.tensor.matmul(p, lhsT=wp_t[:, yc, oc * 128:(oc + 1) * 128],
                                     rhs=yts[tb][:, yc, :],
                                     start=(yc == 0), stop=(yc == 3))
                o = osbp.tile([128, 512], F32, name="osb")
                nc.vector.tensor_scalar_add(out=o, in0=p,
                                            scalar1=bo_t[:, oc:oc + 1])
                nc.sync.dma_start(
                    out=outT[oc * 128:(oc + 1) * 128, tb * 512:(tb + 1) * 512],
                    in_=o)

    nc.compile()
    return nc


def _host_pack(x, W_attn, b_attn, W_proj, b_proj):
    """Per-core input dicts: shard batch (4-way) x head-group (2-way)."""
    B = x.shape[0]
    tri = (np.arange(128)[None, :] >= np.arange(128)[:, None]).astype(np.float32)
    tri = np.ascontiguousarray(tri)
    in_maps = []
    for i in range(N_CORES):
        b, hg = i // 2, i % 2
        h0 = hg * HL
        xT = np.ascontiguousarray(x[b].T)
        wqk = np.empty((C, 1024), np.float32)
        bqk = np.empty((128, 8), np.float32)
        for pr in range(4):
            hA, hB = h0 + 2 * pr, h0 + 2 * pr + 1
            mq, mk = 2 * pr, 2 * pr + 1
            wqk[:, mq * 128:mq * 128 + 64] = W_attn[:, hA * 64:(hA + 1) * 64] * 0.125
            wqk[:, mq * 128 + 64:(mq + 1) * 128] = W_attn[:, hB * 64:(hB + 1) * 64] * 0.125
            wqk[:, mk * 128:mk * 128 + 64] = W_attn[:, C + hA * 64:C + (hA + 1) * 64]
            wqk[:, mk * 128 + 64:(mk + 1) * 128] = W_attn[:, C + hB * 64:C + (hB + 1) * 64]
            bqk[0:64, mq] = b_attn[hA * 64:(hA + 1) * 64] * 0.125
            bqk[64:128, mq] = b_attn[hB * 64:(hB + 1) * 64] * 0.125
            bqk[0:64, mk] = b_attn[C + hA * 64:C + (hA + 1) * 64]
            bqk[64:128, mk] = b_attn[C + hB * 64:C + (hB + 1) * 64]
        wv = np.ascontiguousarray(W_attn[:, 2 * C + h0 * 64:2 * C + (h0 + HL) * 64])
        wp = np.ascontiguousarray(W_proj[h0 * 64:(h0 + HL) * 64, :])
        bv = b_attn[2 * C + h0 * 64:2 * C + (h0 + HL) * 64]
        combined = (bv.astype(np.float64) @ wp.astype(np.float64))
        if hg == 0:
            combined = combined + b_proj
        bo = np.ascontiguousarray(combined.astype(np.float32).reshape(8, 128).T)
        in_maps.append(dict(xT=xT, wqk=wqk, bqk=bqk, wv=wv, wp=wp, bo=bo,
                            trimask=tri, vones=np.ones((128, 32), np.float32)))
    return in_maps


def kernel(x, W_attn, b_attn, W_proj, b_proj):
    global last_results
    import os
    x = np.ascontiguousarray(np.asarray(x, dtype=np.float32))
    W_attn = np.asarray(W_attn, dtype=np.float32)
    b_attn = np.asarray(b_attn, dtype=np.float32)
    W_proj = np.asarray(W_proj, dtype=np.float32)
    b_proj = np.asarray(b_proj, dtype=np.float32)

    if "nc" not in _CACHE:
        _CACHE["nc"] = _build_nc()
    nc = _CACHE["nc"]

    in_maps = _host_pack(x, W_attn, b_attn, W_proj, b_proj)
    trace = bool(os.environ.get("KERNEL_TRACE"))
    res = run_bass_kernel_spmd(nc, in_maps, core_ids=list(range(N_CORES)),
                               trace=trace)
    last_results = res

    B = x.shape[0]
    out = np.empty((B, T, C), np.float32)
    for b in range(B):
        out[b] = (res.results[2 * b]["outT"] + res.results[2 * b + 1]["outT"]).T
    return out


# revision 10
# speedup vs baseline: 1.0306x; 1.0306x over previous
"""Causal multi-head attention block (qkv proj -> causal softmax attention ->
out proj) for Trainium2, distributed over 8 NeuronCores.

Sharding: 4-way data parallel over batch x 2-way tensor parallel over heads
(8 heads per core). Each core computes, for its (batch, head-group):
  - QKV projection directly in transposed-per-head layout (Q^T/K^T [hd, T])
  - causal softmax attention entirely in the "scores transposed" [k, q]
    orientation (row-sums via an appended ones-column in the PV matmul)
  - its partial output projection (row-parallel W_proj split)
Host side packs/shards inputs, and sums the two head-group partials per batch.

All matmuls run as float32r (TF32-like, 1 cycle/row at free-dim >= 256).
"""
import numpy as np
from contextlib import ExitStack

import concourse.bacc as bacc
import concourse.tile as tile
from concourse import mybir
from concourse.alu_op_type import AluOpType
from concourse.bass_utils import run_bass_kernel_spmd

F32 = mybir.dt.float32
F32R = mybir.dt.float32r
EXP = mybir.ActivationFunctionType.Exp

T, C, HD, HL = 2048, 1024, 64, 8   # seq, d_model, head_dim, heads-per-core
N_CORES = 8

_CACHE = {}
last_results = None   # BassKernelResults of the most recent run (for test.py)


def _build_nc():
    nc = bacc.Bacc("TRN2", target_bir_lowering=False, debug=False,
                   num_devices=N_CORES)
    xT = nc.dram_tensor("xT", [C, T], F32R, kind="ExternalInput").ap()
    wqk = nc.dram_tensor("wqk", [C, 1024], F32R, kind="ExternalInput").ap()
    wv = nc.dram_tensor("wv", [C, 512], F32R, kind="ExternalInput").ap()
    wp = nc.dram_tensor("wp", [512, C], F32R, kind="ExternalInput").ap()
    bqk = nc.dram_tensor("bqk", [128, 8], F32, kind="ExternalInput").ap()
    bo = nc.dram_tensor("bo", [128, 8], F32, kind="ExternalInput").ap()
    tri = nc.dram_tensor("trimask", [128, 128], F32R, kind="ExternalInput").ap()
    vones = nc.dram_tensor("vones", [128, 32], F32R, kind="ExternalInput").ap()
    outT = nc.dram_tensor("outT", [C, T], F32, kind="ExternalOutput").ap()

    with tile.TileContext(nc) as tc, ExitStack() as ctx:
        def pool(name, bufs, space="SBUF"):
            return ctx.enter_context(tc.tile_pool(name=name, bufs=bufs, space=space))

        constp = pool("const", 1)
        wqkp = pool("wqkp", 1)
        wvp = pool("wvp", 1)
        ytp = pool("ytp", 1)
        vp = pool("vp", 3)
        qkp = pool("qkp", 2)
        bcp = pool("bcp", 1)
        ptp = pool("ptp", 2)
        osbp = pool("osbp", 2)
        rpp = pool("rpp", 2)
        pp = pool("pp", 2, "PSUM")
        stp = pool("stp", 2, "PSUM")
        yp = pool("yp", 2, "PSUM")
        # opened last so it can be released (LIFO) right after the final
        # projection, freeing its 64KB/partition for the wp tile
        xtp_cm = tc.tile_pool(name="xtp", bufs=1)
        xtp = xtp_cm.__enter__()

        bqk_t = constp.tile([128, 8], F32)
        nc.sync.dma_start(out=bqk_t, in_=bqk)
        # tiny warmup exp so the ACT table-set DMA happens during the input
        # loads instead of stalling the first real softmax tile
        warm = constp.tile([1, 8], F32)
        nc.scalar.activation(warm[0:1, :], bqk_t[0:1, :], EXP)
        bo_t = constp.tile([128, 8], F32)
        nc.sync.dma_start(out=bo_t, in_=bo)
        tri_t = constp.tile([128, 128], F32R)
        nc.sync.dma_start(out=tri_t, in_=tri)

        wqkr = wqk.rearrange("(c p) m -> p c m", p=128)   # [128, 8, 1024]
        wvr = wv.rearrange("(c p) m -> p c m", p=128)     # [128, 8, 512]
        wpr = wp.rearrange("(c p) m -> p c m", p=128)     # [128, 4, 1024]
        # first pair's weights land before the bulk x load so the PE can
        # start projecting as soon as x's first t-block arrives
        wqk_pr0 = wqkp.tile([128, 8, 256], F32R, name="wqkpr")
        nc.sync.dma_start(out=wqk_pr0, in_=wqkr[:, :, 0:256])
        wv_g0 = wvp.tile([128, 8, 256], F32R, name="wvg")
        nc.sync.dma_start(out=wv_g0, in_=wvr[:, :, 0:256])
        # x^T resident: [128, c-chunk, t-block, 512]
        xt = xtp.tile([128, 8, 4, 512], F32R)
        xTr = xT.rearrange("(c p) (tb n) -> p c tb n", p=128, n=512)
        for tb in range(4):
            for c in range(8):
                nc.sync.dma_start(out=xt[:, c, tb], in_=xTr[:, c, tb])

        # y^T staging, one tile per t-block so the out-projection's reads
        # only depend on that t-block's writers: each [128(pair rows), pair, 512]
        yts = [ytp.tile([128, 4, 512], F32R, name=f"yt{tb}") for tb in range(4)]

        v_tiles = {}
        qk_tiles = {}

        def v_group_proj(g):
            """Project V for head pairs 2g, 2g+1 (4 heads, N=256 per matmul)."""
            if g == 0:
                wv_g = wv_g0
            else:
                wv_g = wvp.tile([128, 8, 256], F32R, name="wvg")
                nc.sync.dma_start(out=wv_g, in_=wvr[:, :, g * 256:(g + 1) * 256])
            for prl in (2 * g, 2 * g + 1):
                vt = vp.tile([128, 2, 16, 65], F32R, name="v")
                v_tiles[prl] = vt
                # rowsum ones-column (col 64 of every k-tile)
                nc.sync.dma_start(
                    out=vt[:, 0:2, :, 64:65],
                    in_=vones.rearrange("p (a kt one) -> p a kt one", a=2, one=1))
            for tb in range(4):
                for tt in range(4):
                    p = pp.tile([128, 256], F32, name="pp")
                    for c in range(8):
                        nc.tensor.matmul(p, lhsT=xt[:, c, tb, tt * 128:(tt + 1) * 128],
                                         rhs=wv_g[:, c, :],
                                         start=(c == 0), stop=(c == 7))
                    kt = tb * 4 + tt
                    for j, prl in enumerate((2 * g, 2 * g + 1)):
                        src = p[:, j * 128:(j + 1) * 128].rearrange(
                            "p (a d) -> p a d", a=2)
                        nc.vector.tensor_copy(
                            out=v_tiles[prl][:, 0:2, kt, 0:64], in_=src)

        def qk_proj(pr):
            """Q^T/K^T for pair pr: psum rows = head A (0:64) | head B (64:128)."""
            if pr == 0:
                wqk_pr = wqk_pr0
            else:
                wqk_pr = wqkp.tile([128, 8, 256], F32R, name="wqkpr")
                nc.sync.dma_start(out=wqk_pr,
                                  in_=wqkr[:, :, 2 * pr * 128:(2 * pr + 2) * 128])
            qk = qkp.tile([128, 2, T], F32R, name="qk")
            qk_tiles[pr] = qk
            for tb in range(4):
                for mi in range(2):       # 0 = QQ, 1 = KK
                    p = pp.tile([128, 512], F32, name="pp")
                    for c in range(8):
                        nc.tensor.matmul(p,
                                         lhsT=wqk_pr[:, c, mi * 128:(mi + 1) * 128],
                                         rhs=xt[:, c, tb, :],
                                         start=(c == 0), stop=(c == 7))
                    nc.vector.tensor_scalar_add(
                        out=qk[:, mi, tb * 512:(tb + 1) * 512], in0=p,
                        scalar1=bqk_t[:, 2 * pr + mi: 2 * pr + mi + 1])

        def attention(pr):
            qk = qk_tiles[pr]
            vt = v_tiles[pr]
            for qb in range(4):
                K = 4 * qb + 4
                ys = [yp.tile([65, 512], F32, name="y") for _ in range(2)]
                for kt in range(K):
                    d = kt - 4 * qb
                    lo_mm = 0 if d < 0 else min(128 * d, 256)
                    lo_pv = 0 if d < 0 else 128 * d
                    st = stp.tile([128, 1024], F32, name="st")
                    for a in range(2):
                        po = a * 64
                        # S^T[k, q] = K^T_chunk.T @ Q^T ; heads A/B row-packed
                        nc.tensor.matmul(
                            st[:, a * 512 + lo_mm:(a + 1) * 512],
                            lhsT=qk[po:po + 64, 1, kt * 128:(kt + 1) * 128],
                            rhs=qk[po:po + 64, 0,
                                   qb * 512 + lo_mm:(qb + 1) * 512],
                            start=True, stop=True,
                            tile_position=(po, 0))
                    pt = ptp.tile([128, 1024], F32R, name="pt")
                    nc.scalar.activation(pt[:, lo_mm:1024], st[:, lo_mm:1024], EXP)
                    if d >= 0:
                        for a in range(2):
                            blk = slice(a * 512 + 128 * d, a * 512 + 128 * d + 128)
                            nc.vector.tensor_tensor(out=pt[:, blk], in0=pt[:, blk],
                                                    in1=tri_t, op=AluOpType.mult)
                    for a in range(2):
                        nc.tensor.matmul(
                            ys[a][:, lo_pv:512],
                            lhsT=vt[:, a, kt, :],
                            rhs=pt[:, a * 512 + lo_pv:(a + 1) * 512],
                            start=(kt == 0), stop=(kt == K - 1))
                for a in range(2):
                    rs = rpp.tile([1, 512], F32, name="rs")
                    nc.vector.tensor_copy(out=rs[0:1, :], in_=ys[a][64:65, :])
                    rc = rpp.tile([1, 512], F32, name="rc")
                    nc.vector.reciprocal_approx_fast(out=rc[0:1, :],
                                                     in_=rs[0:1, :])
                    bct = bcp.tile([64, 512], F32, name="bc")
                    nc.gpsimd.partition_broadcast(bct, rc[0:1, :], channels=64)
                    nc.vector.tensor_tensor(
                        out=yts[qb][a * 64:(a + 1) * 64, pr, :],
                        in0=ys[a][0:64, :], in1=bct, op=AluOpType.mult)

        qk_proj(0)
        v_group_proj(0)
        attention(0)
        qk_proj(1)
        v_group_proj(1)
        attention(1)
        qk_proj(2)
        attention(2)
        qk_proj(3)
        xtp_cm.__exit__(None, None, None)   # frees x for the wp tile below
        attention(3)

        wpfp = ctx.enter_context(tc.tile_pool(name="wpfp", bufs=1))
        wp_t = wpfp.tile([128, 4, 1024], F32R, name="wpfull")
        nc.sync.dma_start(out=wp_t, in_=wpr)
        for tb in range(4):
            for oc in range(8):
                p = pp.tile([128, 512], F32, name="pp")
                for yc in range(4):
                    nc.tensor.matmul(p, lhsT=wp_t[:, yc, oc * 128:(oc + 1) * 128],
                                     rhs=yts[tb][:, yc, :],
                                     start=(yc == 0), stop=(yc == 3))
                o = osbp.tile([128, 512], F32, name="osb")
                nc.vector.tensor_scalar_add(out=o, in0=p,
                                            scalar1=bo_t[:, oc:oc + 1])
                nc.sync.dma_start(
                    out=outT[oc * 128:(oc + 1) * 128, tb * 512:(tb + 1) * 512],
                    in_=o)

    nc.compile()
    return nc


def _host_pack(x, W_attn, b_attn, W_proj, b_proj):
    """Per-core input dicts: shard batch (4-way) x head-group (2-way)."""
    B = x.shape[0]
    tri = (np.arange(128)[None, :] >= np.arange(128)[:, None]).astype(np.float32)
    tri = np.ascontiguousarray(tri)
    in_maps = []
    for i in range(N_CORES):
        b, hg = i // 2, i % 2
        h0 = hg * HL
        xT = np.ascontiguousarray(x[b].T)
        wqk = np.empty((C, 1024), np.float32)
        bqk = np.empty((128, 8), np.float32)
        for pr in range(4):
            hA, hB = h0 + 2 * pr, h0 + 2 * pr + 1
            mq, mk = 2 * pr, 2 * pr + 1
            wqk[:, mq * 128:mq * 128 + 64] = W_attn[:, hA * 64:(hA + 1) * 64] * 0.125
            wqk[:, mq * 128 + 64:(mq + 1) * 128] = W_attn[:, hB * 64:(hB + 1) * 64] * 0.125
            wqk[:, mk * 128:mk * 128 + 64] = W_attn[:, C + hA * 64:C + (hA + 1) * 64]
            wqk[:, mk * 128 + 64:(mk + 1) * 128] = W_attn[:, C + hB * 64:C + (hB + 1) * 64]
            bqk[0:64, mq] = b_attn[hA * 64:(hA + 1) * 64] * 0.125
            bqk[64:128, mq] = b_attn[hB * 64:(hB + 1) * 64] * 0.125
            bqk[0:64, mk] = b_attn[C + hA * 64:C + (hA + 1) * 64]
            bqk[64:128, mk] = b_attn[C + hB * 64:C + (hB + 1) * 64]
        wv = np.ascontiguousarray(W_attn[:, 2 * C + h0 * 64:2 * C + (h0 + HL) * 64])
        wp = np.ascontiguousarray(W_proj[h0 * 64:(h0 + HL) * 64, :])
        bv = b_attn[2 * C + h0 * 64:2 * C + (h0 + HL) * 64]
        combined = (bv.astype(np.float64) @ wp.astype(np.float64))
        if hg == 0:
            combined = combined + b_proj
        bo = np.ascontiguousarray(combined.astype(np.float32).reshape(8, 128).T)
        in_maps.append(dict(xT=xT, wqk=wqk, bqk=bqk, wv=wv, wp=wp, bo=bo,
                            trimask=tri, vones=np.ones((128, 32), np.float32)))
    return in_maps


def kernel(x, W_attn, b_attn, W_proj, b_proj):
    global last_results
    import os
    x = np.ascontiguousarray(np.asarray(x, dtype=np.float32))
    W_attn = np.asarray(W_attn, dtype=np.float32)
    b_attn = np.asarray(b_attn, dtype=np.float32)
    W_proj = np.asarray(W_proj, dtype=np.float32)
    b_proj = np.asarray(b_proj, dtype=np.float32)

    if "nc" not in _CACHE:
        _CACHE["nc"] = _build_nc()
    nc = _CACHE["nc"]

    in_maps = _host_pack(x, W_attn, b_attn, W_proj, b_proj)
    trace = bool(os.environ.get("KERNEL_TRACE"))
    res = run_bass_kernel_spmd(nc, in_maps, core_ids=list(range(N_CORES)),
                               trace=trace)
    last_results = res

    B = x.shape[0]
    out = np.empty((B, T, C), np.float32)
    for b in range(B):
        out[b] = (res.results[2 * b]["outT"] + res.results[2 * b + 1]["outT"]).T
    return out


# revision 13
# speedup vs baseline: 1.0369x; 1.0061x over previous
"""Causal multi-head attention block (qkv proj -> causal softmax attention ->
out proj) for Trainium2, distributed over 8 NeuronCores.

Sharding: 4-way data parallel over batch x 2-way tensor parallel over heads
(8 heads per core). Each core computes, for its (batch, head-group):
  - QKV projection directly in transposed-per-head layout (Q^T/K^T [hd, T])
  - causal softmax attention entirely in the "scores transposed" [k, q]
    orientation (row-sums via an appended ones-column in the PV matmul)
  - its partial output projection (row-parallel W_proj split)
Host side packs/shards inputs, and sums the two head-group partials per batch.

All matmuls run as float32r (TF32-like, 1 cycle/row at free-dim >= 256).
"""
import numpy as np
from contextlib import ExitStack

import concourse.bacc as bacc
import concourse.tile as tile
from concourse import mybir
from concourse.alu_op_type import AluOpType
from concourse.bass_utils import run_bass_kernel_spmd

F32 = mybir.dt.float32
F32R = mybir.dt.float32r
EXP = mybir.ActivationFunctionType.Exp

T, C, HD, HL = 2048, 1024, 64, 8   # seq, d_model, head_dim, heads-per-core
N_CORES = 8

_CACHE = {}
last_results = None   # BassKernelResults of the most recent run (for test.py)


def _build_nc():
    nc = bacc.Bacc("TRN2", target_bir_lowering=False, debug=False,
                   num_devices=N_CORES)
    xT = nc.dram_tensor("xT", [C, T], F32R, kind="ExternalInput").ap()
    wqk = nc.dram_tensor("wqk", [C, 1024], F32R, kind="ExternalInput").ap()
    wv = nc.dram_tensor("wv", [C, 512], F32R, kind="ExternalInput").ap()
    wp = nc.dram_tensor("wp", [512, C], F32R, kind="ExternalInput").ap()
    bqk = nc.dram_tensor("bqk", [128, 8], F32, kind="ExternalInput").ap()
    bo = nc.dram_tensor("bo", [128, 8], F32, kind="ExternalInput").ap()
    tri = nc.dram_tensor("trimask", [128, 128], F32R, kind="ExternalInput").ap()
    vones = nc.dram_tensor("vones", [128, 32], F32R, kind="ExternalInput").ap()
    outT = nc.dram_tensor("outT", [C, T], F32, kind="ExternalOutput").ap()

    with tile.TileContext(nc) as tc, ExitStack() as ctx:
        def pool(name, bufs, space="SBUF"):
            return ctx.enter_context(tc.tile_pool(name=name, bufs=bufs, space=space))

        constp = pool("const", 1)
        wqkp = pool("wqkp", 1)
        wvp = pool("wvp", 1)
        ytp = pool("ytp", 1)
        vp = pool("vp", 3)
        qkp = pool("qkp", 2)
        bcp = pool("bcp", 1)
        ptp = pool("ptp", 2)
        osbp = pool("osbp", 2)
        rpp = pool("rpp", 2)
        pp = pool("pp", 2, "PSUM")
        stp = pool("stp", 2, "PSUM")
        yp = pool("yp", 2, "PSUM")
        # opened last so it can be released (LIFO) right after the final
        # projection, freeing its 64KB/partition for the wp tile
        xtp_cm = tc.tile_pool(name="xtp", bufs=1)
        xtp = xtp_cm.__enter__()

        bqk_t = constp.tile([128, 8], F32)
        nc.sync.dma_start(out=bqk_t, in_=bqk)
        # tiny warmup exp so the ACT table-set DMA happens during the input
        # loads instead of stalling the first real softmax tile
        warm = constp.tile([1, 8], F32)
        nc.scalar.activation(warm[0:1, :], bqk_t[0:1, :], EXP)
        bo_t = constp.tile([128, 8], F32)
        nc.sync.dma_start(out=bo_t, in_=bo)
        tri_t = constp.tile([128, 128], F32R)
        nc.sync.dma_start(out=tri_t, in_=tri)

        wqkr = wqk.rearrange("(c p) m -> p c m", p=128)   # [128, 8, 1024]
        wvr = wv.rearrange("(c p) m -> p c m", p=128)     # [128, 8, 512]
        wpr = wp.rearrange("(c p) m -> p c m", p=128)     # [128, 4, 1024]
        # first pair's weights land before the bulk x load so the PE can
        # start projecting as soon as x's first t-block arrives
        wqk_pr0 = wqkp.tile([128, 8, 256], F32R, name="wqkpr")
        nc.sync.dma_start(out=wqk_pr0, in_=wqkr[:, :, 0:256])
        # x^T resident: [128, c-chunk, t-block, 512]; first t-block lands
        # before wv so the first QK projection starts as early as possible
        xt = xtp.tile([128, 8, 4, 512], F32R)
        xTr = xT.rearrange("(c p) (tb n) -> p c tb n", p=128, n=512)
        for c in range(8):
            nc.sync.dma_start(out=xt[:, c, 0], in_=xTr[:, c, 0])
        wv_g0 = wvp.tile([128, 8, 256], F32R, name="wvg")
        nc.sync.dma_start(out=wv_g0, in_=wvr[:, :, 0:256])
        for tb in range(1, 4):
            for c in range(8):
                nc.sync.dma_start(out=xt[:, c, tb], in_=xTr[:, c, tb])

        # y^T staging, one tile per t-block so the out-projection's reads
        # only depend on that t-block's writers: each [128(pair rows), pair, 512]
        yts = [ytp.tile([128, 4, 512], F32R, name=f"yt{tb}") for tb in range(4)]

        v_tiles = {}
        qk_tiles = {}

        def v_group_proj(g):
            """Project V for head pairs 2g, 2g+1 (4 heads, N=256 per matmul)."""
            if g == 0:
                wv_g = wv_g0
            else:
                wv_g = wvp.tile([128, 8, 256], F32R, name="wvg")
                nc.sync.dma_start(out=wv_g, in_=wvr[:, :, g * 256:(g + 1) * 256])
            for prl in (2 * g, 2 * g + 1):
                vt = vp.tile([128, 2, 16, 65], F32R, name="v")
                v_tiles[prl] = vt
                # rowsum ones-column (col 64 of every k-tile)
                nc.sync.dma_start(
                    out=vt[:, 0:2, :, 64:65],
                    in_=vones.rearrange("p (a kt one) -> p a kt one", a=2, one=1))
            for tb in range(4):
                for tt in range(4):
                    p = pp.tile([128, 256], F32, name="pp")
                    for c in range(8):
                        nc.tensor.matmul(p, lhsT=xt[:, c, tb, tt * 128:(tt + 1) * 128],
                                         rhs=wv_g[:, c, :],
                                         start=(c == 0), stop=(c == 7))
                    kt = tb * 4 + tt
                    for j, prl in enumerate((2 * g, 2 * g + 1)):
                        src = p[:, j * 128:(j + 1) * 128].rearrange(
                            "p (a d) -> p a d", a=2)
                        nc.vector.tensor_copy(
                            out=v_tiles[prl][:, 0:2, kt, 0:64], in_=src)

        def qk_proj(pr):
            """Q^T/K^T for pair pr: psum rows = head A (0:64) | head B (64:128)."""
            if pr == 0:
                wqk_pr = wqk_pr0
            else:
                wqk_pr = wqkp.tile([128, 8, 256], F32R, name="wqkpr")
                nc.sync.dma_start(out=wqk_pr,
                                  in_=wqkr[:, :, 2 * pr * 128:(2 * pr + 2) * 128])
            qk = qkp.tile([128, 2, T], F32R, name="qk")
            qk_tiles[pr] = qk
            for tb in range(4):
                for mi in range(2):       # 0 = QQ, 1 = KK
                    p = pp.tile([128, 512], F32, name="pp")
                    for c in range(8):
                        nc.tensor.matmul(p,
                                         lhsT=wqk_pr[:, c, mi * 128:(mi + 1) * 128],
                                         rhs=xt[:, c, tb, :],
                                         start=(c == 0), stop=(c == 7))
                    nc.vector.tensor_scalar_add(
                        out=qk[:, mi, tb * 512:(tb + 1) * 512], in0=p,
                        scalar1=bqk_t[:, 2 * pr + mi: 2 * pr + mi + 1])

        def attention(pr):
            qk = qk_tiles[pr]
            vt = v_tiles[pr]
            for qb in range(4):
                K = 4 * qb + 4
                ys = [yp.tile([65, 512], F32, name="y") for _ in range(2)]
                kts = list(range(K))
                if qb >= 1:
                    kts[0], kts[1] = kts[1], kts[0]   # kt=1 (full) starts the
                    # PSUM accumulation group; kt=0 (also full) accumulates later
                first_kt = kts[0]
                for kt in kts:
                    d = kt - 4 * qb
                    lo_mm = 0 if d < 0 else min(128 * d, 256)
                    lo_pv = 0 if d < 0 else 128 * d
                    st = stp.tile([128, 1024], F32, name="st")
                    for a in range(2):
                        po = a * 64
                        # S^T[k, q] = K^T_chunk.T @ Q^T ; heads A/B row-packed
                        nc.tensor.matmul(
                            st[:, a * 512 + lo_mm:(a + 1) * 512],
                            lhsT=qk[po:po + 64, 1, kt * 128:(kt + 1) * 128],
                            rhs=qk[po:po + 64, 0,
                                   qb * 512 + lo_mm:(qb + 1) * 512],
                            start=True, stop=True,
                            tile_position=(po, 0))
                    pt = ptp.tile([128, 1024], F32R, name="pt")
                    nc.scalar.activation(pt[:, lo_mm:1024], st[:, lo_mm:1024], EXP)
                    if d >= 0:
                        for a in range(2):
                            blk = slice(a * 512 + 128 * d, a * 512 + 128 * d + 128)
                            nc.vector.tensor_tensor(out=pt[:, blk], in0=pt[:, blk],
                                                    in1=tri_t, op=AluOpType.mult)
                    for a in range(2):
                        nc.tensor.matmul(
                            ys[a][:, lo_pv:512],
                            lhsT=vt[:, a, kt, :],
                            rhs=pt[:, a * 512 + lo_pv:(a + 1) * 512],
                            start=(kt == first_kt), stop=(kt == K - 1))
                for a in range(2):
                    rs = rpp.tile([1, 512], F32, name="rs")
                    nc.vector.tensor_copy(out=rs[0:1, :], in_=ys[a][64:65, :])
                    rc = rpp.tile([1, 512], F32, name="rc")
                    nc.vector.reciprocal_approx_fast(out=rc[0:1, :],
                                                     in_=rs[0:1, :])
                    bct = bcp.tile([64, 512], F32, name="bc")
                    nc.gpsimd.partition_broadcast(bct, rc[0:1, :], channels=64)
                    nc.vector.tensor_tensor(
                        out=yts[qb][a * 64:(a + 1) * 64, pr, :],
                        in0=ys[a][0:64, :], in1=bct, op=AluOpType.mult)

        qk_proj(0)
        v_group_proj(0)
        attention(0)
        qk_proj(1)
        v_group_proj(1)
        attention(1)
        qk_proj(2)
        attention(2)
        qk_proj(3)
        xtp_cm.__exit__(None, None, None)   # frees x for the wp tile below
        attention(3)

        wpfp = ctx.enter_context(tc.tile_pool(name="wpfp", bufs=1))
        wp_t = wpfp.tile([128, 4, 1024], F32R, name="wpfull")
        nc.sync.dma_start(out=wp_t, in_=wpr)
        for tb in range(4):
            for oc in range(8):
                p = pp.tile([128, 512], F32, name="pp")
                for yc in range(4):
                    nc.tensor.matmul(p, lhsT=wp_t[:, yc, oc * 128:(oc + 1) * 128],
                                     rhs=yts[tb][:, yc, :],
                                     start=(yc == 0), stop=(yc == 3))
                o = osbp.tile([128, 512], F32, name="osb")
                nc.vector.tensor_scalar_add(out=o, in0=p,
                                            scalar1=bo_t[:, oc:oc + 1])
                nc.sync.dma_start(
                    out=outT[oc * 128:(oc + 1) * 128, tb * 512:(tb + 1) * 512],
                    in_=o)

    nc.compile()
    return nc


def _host_pack(x, W_attn, b_attn, W_proj, b_proj):
    """Per-core input dicts: shard batch (4-way) x head-group (2-way)."""
    B = x.shape[0]
    tri = (np.arange(128)[None, :] >= np.arange(128)[:, None]).astype(np.float32)
    tri = np.ascontiguousarray(tri)
    in_maps = []
    for i in range(N_CORES):
        b, hg = i // 2, i % 2
        h0 = hg * HL
        xT = np.ascontiguousarray(x[b].T)
        wqk = np.empty((C, 1024), np.float32)
        bqk = np.empty((128, 8), np.float32)
        for pr in range(4):
            hA, hB = h0 + 2 * pr, h0 + 2 * pr + 1
            mq, mk = 2 * pr, 2 * pr + 1
            wqk[:, mq * 128:mq * 128 + 64] = W_attn[:, hA * 64:(hA + 1) * 64] * 0.125
            wqk[:, mq * 128 + 64:(mq + 1) * 128] = W_attn[:, hB * 64:(hB + 1) * 64] * 0.125
            wqk[:, mk * 128:mk * 128 + 64] = W_attn[:, C + hA * 64:C + (hA + 1) * 64]
            wqk[:, mk * 128 + 64:(mk + 1) * 128] = W_attn[:, C + hB * 64:C + (hB + 1) * 64]
            bqk[0:64, mq] = b_attn[hA * 64:(hA + 1) * 64] * 0.125
            bqk[64:128, mq] = b_attn[hB * 64:(hB + 1) * 64] * 0.125
            bqk[0:64, mk] = b_attn[C + hA * 64:C + (hA + 1) * 64]
            bqk[64:128, mk] = b_attn[C + hB * 64:C + (hB + 1) * 64]
        wv = np.ascontiguousarray(W_attn[:, 2 * C + h0 * 64:2 * C + (h0 + HL) * 64])
        wp = np.ascontiguousarray(W_proj[h0 * 64:(h0 + HL) * 64, :])
        bv = b_attn[2 * C + h0 * 64:2 * C + (h0 + HL) * 64]
        combined = (bv.astype(np.float64) @ wp.astype(np.float64))
        if hg == 0:
            combined = combined + b_proj
        bo = np.ascontiguousarray(combined.astype(np.float32).reshape(8, 128).T)
        in_maps.append(dict(xT=xT, wqk=wqk, bqk=bqk, wv=wv, wp=wp, bo=bo,
                            trimask=tri, vones=np.ones((128, 32), np.float32)))
    return in_maps


def kernel(x, W_attn, b_attn, W_proj, b_proj):
    global last_results
    import os
    x = np.ascontiguousarray(np.asarray(x, dtype=np.float32))
    W_attn = np.asarray(W_attn, dtype=np.float32)
    b_attn = np.asarray(b_attn, dtype=np.float32)
    W_proj = np.asarray(W_proj, dtype=np.float32)
    b_proj = np.asarray(b_proj, dtype=np.float32)

    if "nc" not in _CACHE:
        _CACHE["nc"] = _build_nc()
    nc = _CACHE["nc"]

    in_maps = _host_pack(x, W_attn, b_attn, W_proj, b_proj)
    trace = bool(os.environ.get("KERNEL_TRACE"))
    res = run_bass_kernel_spmd(nc, in_maps, core_ids=list(range(N_CORES)),
                               trace=trace)
    last_results = res

    B = x.shape[0]
    out = np.empty((B, T, C), np.float32)
    for b in range(B):
        out[b] = (res.results[2 * b]["outT"] + res.results[2 * b + 1]["outT"]).T
    return out


# revision 14
# speedup vs baseline: 1.1095x; 1.0701x over previous
"""Causal multi-head attention block (qkv proj -> causal softmax attention ->
out proj) for Trainium2, distributed over 8 NeuronCores.

Sharding: 4-way data parallel over batch x 2-way tensor parallel over heads
(8 heads per core). Each core computes, for its (batch, head-group):
  - QKV projection directly in transposed-per-head layout (Q^T/K^T [hd, T])
  - causal softmax attention entirely in the "scores transposed" [k, q]
    orientation (row-sums via an appended ones-column in the PV matmul)
  - its partial output projection (row-parallel W_proj split)
Host side packs/shards inputs, and sums the two head-group partials per batch.

All matmuls run as float32r (TF32-like, 1 cycle/row at free-dim >= 256).
"""
import numpy as np
from contextlib import ExitStack

import concourse.bacc as bacc
import concourse.tile as tile
from concourse import mybir
from concourse.alu_op_type import AluOpType
from concourse.bass_utils import run_bass_kernel_spmd

F32 = mybir.dt.float32
F32R = mybir.dt.float32r
EXP = mybir.ActivationFunctionType.Exp

T, C, HD, HL = 2048, 1024, 64, 8   # seq, d_model, head_dim, heads-per-core
N_CORES = 8

_CACHE = {}
last_results = None   # BassKernelResults of the most recent run (for test.py)


def _build_nc():
    nc = bacc.Bacc("TRN2", target_bir_lowering=False, debug=False,
                   num_devices=N_CORES)
    xT = nc.dram_tensor("xT", [C, T], F32R, kind="ExternalInput").ap()
    wqk = nc.dram_tensor("wqk", [C, 1024], F32R, kind="ExternalInput").ap()
    wv = nc.dram_tensor("wv", [C, 512], F32R, kind="ExternalInput").ap()
    wp = nc.dram_tensor("wp", [512, C], F32R, kind="ExternalInput").ap()
    bqk = nc.dram_tensor("bqk", [128, 8], F32, kind="ExternalInput").ap()
    bo = nc.dram_tensor("bo", [128, 8], F32, kind="ExternalInput").ap()
    tri = nc.dram_tensor("trimask", [128, 128], F32R, kind="ExternalInput").ap()
    vones = nc.dram_tensor("vones", [128, 32], F32R, kind="ExternalInput").ap()
    outT = nc.dram_tensor("outT", [C, T], F32, kind="ExternalOutput").ap()

    with tile.TileContext(nc) as tc, ExitStack() as ctx:
        def pool(name, bufs, space="SBUF"):
            return ctx.enter_context(tc.tile_pool(name=name, bufs=bufs, space=space))

        constp = pool("const", 1)
        wqkp = pool("wqkp", 1)
        wvp = pool("wvp", 1)
        ytp = pool("ytp", 1)
        vp = pool("vp", 3)
        qkp = pool("qkp", 2)
        bcp = pool("bcp", 1)
        ptp = pool("ptp", 3)
        osbp = pool("osbp", 2)
        rpp = pool("rpp", 1)
        pp = pool("pp", 2, "PSUM")
        stp = pool("stp", 2, "PSUM")
        yp = pool("yp", 2, "PSUM")
        # opened last so it can be released (LIFO) right after the final
        # projection, freeing its 64KB/partition for the wp tile
        xtp_cm = tc.tile_pool(name="xtp", bufs=1)
        xtp = xtp_cm.__enter__()

        bqk_t = constp.tile([128, 8], F32)
        nc.sync.dma_start(out=bqk_t, in_=bqk)
        # tiny warmup exp so the ACT table-set DMA happens during the input
        # loads instead of stalling the first real softmax tile
        warm = constp.tile([1, 8], F32)
        nc.scalar.activation(warm[0:1, :], bqk_t[0:1, :], EXP)
        bo_t = constp.tile([128, 8], F32)
        nc.sync.dma_start(out=bo_t, in_=bo)
        tri_t = constp.tile([128, 128], F32R)
        nc.sync.dma_start(out=tri_t, in_=tri)

        wqkr = wqk.rearrange("(c p) m -> p c m", p=128)   # [128, 8, 1024]
        wvr = wv.rearrange("(c p) m -> p c m", p=128)     # [128, 8, 512]
        wpr = wp.rearrange("(c p) m -> p c m", p=128)     # [128, 4, 1024]
        # first pair's weights land before the bulk x load so the PE can
        # start projecting as soon as x's first t-block arrives
        wqk_pr0 = wqkp.tile([128, 8, 256], F32R, name="wqkpr")
        nc.sync.dma_start(out=wqk_pr0, in_=wqkr[:, :, 0:256])
        # x^T resident: [128, c-chunk, t-block, 512]; first t-block lands
        # before wv so the first QK projection starts as early as possible
        xt = xtp.tile([128, 8, 4, 512], F32R)
        xTr = xT.rearrange("(c p) (tb n) -> p c tb n", p=128, n=512)
        for c in range(8):
            nc.sync.dma_start(out=xt[:, c, 0], in_=xTr[:, c, 0])
        wv_g0 = wvp.tile([128, 8, 256], F32R, name="wvg")
        nc.sync.dma_start(out=wv_g0, in_=wvr[:, :, 0:256])
        for tb in range(1, 4):
            for c in range(8):
                nc.sync.dma_start(out=xt[:, c, tb], in_=xTr[:, c, tb])

        # y^T staging, one tile per t-block so the out-projection's reads
        # only depend on that t-block's writers: each [128(pair rows), pair, 512]
        yts = [ytp.tile([128, 4, 512], F32R, name=f"yt{tb}") for tb in range(4)]

        v_tiles = {}
        qk_tiles = {}

        def v_group_proj(g):
            """Project V for head pairs 2g, 2g+1 (4 heads, N=256 per matmul)."""
            if g == 0:
                wv_g = wv_g0
            else:
                wv_g = wvp.tile([128, 8, 256], F32R, name="wvg")
                nc.sync.dma_start(out=wv_g, in_=wvr[:, :, g * 256:(g + 1) * 256])
            for prl in (2 * g, 2 * g + 1):
                vt = vp.tile([128, 2, 16, 65], F32R, name="v")
                v_tiles[prl] = vt
                # rowsum ones-column (col 64 of every k-tile)
                nc.sync.dma_start(
                    out=vt[:, 0:2, :, 64:65],
                    in_=vones.rearrange("p (a kt one) -> p a kt one", a=2, one=1))
            for tb in range(4):
                for tt in range(4):
                    p = pp.tile([128, 256], F32, name="pp")
                    for c in range(8):
                        nc.tensor.matmul(p, lhsT=xt[:, c, tb, tt * 128:(tt + 1) * 128],
                                         rhs=wv_g[:, c, :],
                                         start=(c == 0), stop=(c == 7))
                    kt = tb * 4 + tt
                    for j, prl in enumerate((2 * g, 2 * g + 1)):
                        src = p[:, j * 128:(j + 1) * 128].rearrange(
                            "p (a d) -> p a d", a=2)
                        nc.vector.tensor_copy(
                            out=v_tiles[prl][:, 0:2, kt, 0:64], in_=src)

        def qk_proj(pr):
            """Q^T/K^T for pair pr: psum rows = head A (0:64) | head B (64:128)."""
            if pr == 0:
                wqk_pr = wqk_pr0
            else:
                wqk_pr = wqkp.tile([128, 8, 256], F32R, name="wqkpr")
                nc.sync.dma_start(out=wqk_pr,
                                  in_=wqkr[:, :, 2 * pr * 128:(2 * pr + 2) * 128])
            qk = qkp.tile([128, 2, T], F32R, name="qk")
            qk_tiles[pr] = qk
            for tb in range(4):
                for mi in range(2):       # 0 = QQ, 1 = KK
                    p = pp.tile([128, 512], F32, name="pp")
                    for c in range(8):
                        nc.tensor.matmul(p,
                                         lhsT=wqk_pr[:, c, mi * 128:(mi + 1) * 128],
                                         rhs=xt[:, c, tb, :],
                                         start=(c == 0), stop=(c == 7))
                    nc.vector.tensor_scalar_add(
                        out=qk[:, mi, tb * 512:(tb + 1) * 512], in0=p,
                        scalar1=bqk_t[:, 2 * pr + mi: 2 * pr + mi + 1])

        def attention(pr):
            qk = qk_tiles[pr]
            vt = v_tiles[pr]
            for qb in range(4):
                K = 4 * qb + 4
                ys = [yp.tile([65, 512], F32, name="y") for _ in range(2)]
                kts = list(range(K))
                if qb >= 1:
                    kts[0], kts[1] = kts[1], kts[0]   # kt=1 (full) starts the
                    # PSUM accumulation group; kt=0 (also full) accumulates later
                first_kt = kts[0]
                for kt in kts:
                    d = kt - 4 * qb
                    lo_mm = 0 if d < 0 else min(128 * d, 256)
                    lo_pv = 0 if d < 0 else 128 * d
                    st = stp.tile([128, 1024], F32, name="st")
                    for a in range(2):
                        po = a * 64
                        # S^T[k, q] = K^T_chunk.T @ Q^T ; heads A/B row-packed
                        nc.tensor.matmul(
                            st[:, a * 512 + lo_mm:(a + 1) * 512],
                            lhsT=qk[po:po + 64, 1, kt * 128:(kt + 1) * 128],
                            rhs=qk[po:po + 64, 0,
                                   qb * 512 + lo_mm:(qb + 1) * 512],
                            start=True, stop=True,
                            tile_position=(po, 0))
                    pt = ptp.tile([128, 1024], F32R, name="pt")
                    nc.scalar.activation(pt[:, lo_mm:1024], st[:, lo_mm:1024], EXP)
                    if d >= 0:
                        for a in range(2):
                            blk = slice(a * 512 + 128 * d, a * 512 + 128 * d + 128)
                            nc.vector.tensor_tensor(out=pt[:, blk], in0=pt[:, blk],
                                                    in1=tri_t, op=AluOpType.mult)
                    for a in range(2):
                        nc.tensor.matmul(
                            ys[a][:, lo_pv:512],
                            lhsT=vt[:, a, kt, :],
                            rhs=pt[:, a * 512 + lo_pv:(a + 1) * 512],
                            start=(kt == first_kt), stop=(kt == K - 1))
                for a in range(2):
                    rs = rpp.tile([1, 512], F32, name="rs")
                    nc.vector.tensor_copy(out=rs[0:1, :], in_=ys[a][64:65, :])
                    rc = rpp.tile([1, 512], F32, name="rc")
                    nc.vector.reciprocal_approx_fast(out=rc[0:1, :],
                                                     in_=rs[0:1, :])
                    bct = bcp.tile([64, 512], F32, name="bc")
                    nc.gpsimd.partition_broadcast(bct, rc[0:1, :], channels=64)
                    nc.vector.tensor_tensor(
                        out=yts[qb][a * 64:(a + 1) * 64, pr, :],
                        in0=ys[a][0:64, :], in1=bct, op=AluOpType.mult)

        qk_proj(0)
        v_group_proj(0)
        attention(0)
        qk_proj(1)
        v_group_proj(1)
        attention(1)
        qk_proj(2)
        attention(2)
        qk_proj(3)
        xtp_cm.__exit__(None, None, None)   # frees x for the wp tile below
        attention(3)

        wpfp = ctx.enter_context(tc.tile_pool(name="wpfp", bufs=1))
        wp_t = wpfp.tile([128, 4, 1024], F32R, name="wpfull")
        nc.sync.dma_start(out=wp_t, in_=wpr)
        for tb in range(4):
            for oc in range(8):
                p = pp.tile([128, 512], F32, name="pp")
                for yc in range(4):
                    nc.tensor.matmul(p, lhsT=wp_t[:, yc, oc * 128:(oc + 1) * 128],
                                     rhs=yts[tb][:, yc, :],
                                     start=(yc == 0), stop=(yc == 3))
                o = osbp.tile([128, 512], F32, name="osb")
                nc.vector.tensor_scalar_add(out=o, in0=p,
                                            scalar1=bo_t[:, oc:oc + 1])
                nc.sync.dma_start(
                    out=outT[oc * 128:(oc + 1) * 128, tb * 512:(tb + 1) * 512],
                    in_=o)

    nc.compile()
    return nc


def _host_pack(x, W_attn, b_attn, W_proj, b_proj):
    """Per-core input dicts: shard batch (4-way) x head-group (2-way)."""
    B = x.shape[0]
    tri = (np.arange(128)[None, :] >= np.arange(128)[:, None]).astype(np.float32)
    tri = np.ascontiguousarray(tri)
    in_maps = []
    for i in range(N_CORES):
        b, hg = i // 2, i % 2
        h0 = hg * HL
        xT = np.ascontiguousarray(x[b].T)
        wqk = np.empty((C, 1024), np.float32)
        bqk = np.empty((128, 8), np.float32)
        for pr in range(4):
            hA, hB = h0 + 2 * pr, h0 + 2 * pr + 1
            mq, mk = 2 * pr, 2 * pr + 1
            wqk[:, mq * 128:mq * 128 + 64] = W_attn[:, hA * 64:(hA + 1) * 64] * 0.125
            wqk[:, mq * 128 + 64:(mq + 1) * 128] = W_attn[:, hB * 64:(hB + 1) * 64] * 0.125
            wqk[:, mk * 128:mk * 128 + 64] = W_attn[:, C + hA * 64:C + (hA + 1) * 64]
            wqk[:, mk * 128 + 64:(mk + 1) * 128] = W_attn[:, C + hB * 64:C + (hB + 1) * 64]
            bqk[0:64, mq] = b_attn[hA * 64:(hA + 1) * 64] * 0.125
            bqk[64:128, mq] = b_attn[hB * 64:(hB + 1) * 64] * 0.125
            bqk[0:64, mk] = b_attn[C + hA * 64:C + (hA + 1) * 64]
            bqk[64:128, mk] = b_attn[C + hB * 64:C + (hB + 1) * 64]
        wv = np.ascontiguousarray(W_attn[:, 2 * C + h0 * 64:2 * C + (h0 + HL) * 64])
        wp = np.ascontiguousarray(W_proj[h0 * 64:(h0 + HL) * 64, :])
        bv = b_attn[2 * C + h0 * 64:2 * C + (h0 + HL) * 64]
        combined = (bv.astype(np.float64) @ wp.astype(np.float64))
        if hg == 0:
            combined = combined + b_proj
        bo = np.ascontiguousarray(combined.astype(np.float32).reshape(8, 128).T)
        in_maps.append(dict(xT=xT, wqk=wqk, bqk=bqk, wv=wv, wp=wp, bo=bo,
                            trimask=tri, vones=np.ones((128, 32), np.float32)))
    return in_maps


def kernel(x, W_attn, b_attn, W_proj, b_proj):
    global last_results
    import os
    x = np.ascontiguousarray(np.asarray(x, dtype=np.float32))
    W_attn = np.asarray(W_attn, dtype=np.float32)
    b_attn = np.asarray(b_attn, dtype=np.float32)
    W_proj = np.asarray(W_proj, dtype=np.float32)
    b_proj = np.asarray(b_proj, dtype=np.float32)

    if "nc" not in _CACHE:
        _CACHE["nc"] = _build_nc()
    nc = _CACHE["nc"]

    in_maps = _host_pack(x, W_attn, b_attn, W_proj, b_proj)
    trace = bool(os.environ.get("KERNEL_TRACE"))
    res = run_bass_kernel_spmd(nc, in_maps, core_ids=list(range(N_CORES)),
                               trace=trace)
    last_results = res

    B = x.shape[0]
    out = np.empty((B, T, C), np.float32)
    for b in range(B):
        out[b] = (res.results[2 * b]["outT"] + res.results[2 * b + 1]["outT"]).T
    return out
